# revision 15
# baseline (speedup 1.0000x reference)
"""Non-local (dot-product attention) block kernel for Trainium2, 8 cores.

Reference math (per sample):
    t = theta_w @ X + bt 1^T            (D, N)
    p = phi_w @ X + bp 1^T              (D, N)
    g = g_w @ X + bg 1^T                (D, N)
    f = t^T p / N;  y^T = g f^T;  z = BN(w_w y^T) + x

Gram-form collapse (no softmax => pure matmul associativity), with the
D-sized weight chain contracted ON THE HOST into two C x C matrices:
    S   = X X^T, sx = X 1 (ones column)                    (C, C), (C,)
    Q   = g_w^T w''^T,  P2 = phi_w^T theta_w               (host, C x C)
    T2w = I + P2^T (S Q) + tb2 (Q^T sx + N hb)^T + (P2^T sx) hb^T
    b2  = Q^T (S (t1 + sbb)) + (sx . t1) hb + bc           (host vectors)
    z   = T2w^T X + b2 1^T
where w'' = diag(inv) w_w / N, tb2 = theta_w^T bp, hb = w'' bg,
t1 = phi_w^T bt, sbb = bp.bt, bc = N sbb hb + b'.

So the device work is: one C x C x N Gram, a 2-hop C x C chain
(S -> SQ -> T2w, plus tiny row/column bias matmuls), and one C x C x N
output matmul. Everything else is folded on the host.

Device plan per core (data-parallel, one sample per core, no collectives):
  - The PE is power-throttled to ~50% duty under sustained load, so total
    matmul columns are minimized: the Gram exploits symmetry (S1 block
    only computes cols [128:257]; the missing S10 block is one PE
    transpose of S01), and the weight chain is tiny.
  - Per-queue DMA tops out well under the ~360 GB/s core aggregate, so xt
    is interleaved across BOTH HWDGE rings (small first pieces); the xc
    halves queue behind xt, and the packed weights ride the gpsimd SWDGE
    queue so they never block the rings.
  - The PE clock also ramps with activity (~2x slower cold), so a few
    warmup matmuls on junk data run while the first xt piece lands.
  - Output matmuls interleave the two c-half PSUM banks (same-bank
    back-to-back matmuls stall ~2x).
  - PSUM bank budget (8): S banks are reused for T2w + the bias chain
    (disjoint columns), SQ banks carry the qsx/psx rows, and the z phase
    cycles 6 banks (4 dedicated + the retired SQ banks).
"""

import numpy as np

B, C, HH, WW = 8, 256, 96, 32
N = HH * WW          # 3072
D = 128              # inter_channels
BN_EPS = 1e-5
NT = N // 128        # 24 pixel chunks
N_CORES = 8

_NC = None

# xt pieces (chunk_start, n_chunks, ring): ring 0 = sync, ring 1 = scalar
XT_PIECES = [(0, 1, 0), (1, 1, 1), (2, 3, 0), (5, 3, 1),
             (8, 4, 0), (12, 4, 1), (16, 4, 0), (20, 4, 1)]
Z_ORDER = [0, 1, 2, 3, 4, 5]
N_WARMUP = 4


def _build_nc():
    from contextlib import ExitStack

    import concourse.bass as bass
    import concourse.bacc as bacc
    import concourse.tile as tile
    from concourse import mybir

    f32 = mybir.dt.float32
    f16 = mybir.dt.bfloat16
    AF = mybir.ActivationFunctionType
    ALU = mybir.AluOpType

    nc = bacc.Bacc(
        "TRN2",
        target_bir_lowering=False,
        debug=False,
        num_devices=N_CORES,
    )

    # xt: 24 blocks of 257 cols: block i = x^T[128i:128(i+1), :] | ones
    xt = nc.dram_tensor("xt", [128, NT * 257], f16, kind="ExternalInput").ap()
    # xc: 6 blocks of 1024: block j = [x[0:128, 512j:512j+512] | x[128:256, ...]]
    xc = nc.dram_tensor("xc", [128, 6144], f16, kind="ExternalInput").ap()
    # wpk f16 cols: Q0 256 | Q1 256 | P2q 512 | I128 128 | t1c 2 | t1p 2
    wpk = nc.dram_tensor("wpk", [128, 578], f32, kind="ExternalInput").ap()
    # aux f16 cols: tb2 256 | hb 256 | N*hb 256 | bc 256 | one 1 | pad
    aux = nc.dram_tensor("aux", [1, 514], f32, kind="ExternalInput").ap()
    # out: [z[0:128, :] | z[128:256, :]] fp16
    out = nc.dram_tensor("out", [128, 6144], f16, kind="ExternalOutput").ap()

    with tile.TileContext(nc) as tc, ExitStack() as ctx:
        const = ctx.enter_context(tc.tile_pool(name="const", bufs=1))
        zpool = ctx.enter_context(tc.tile_pool(name="zpool", bufs=4))
        psS = ctx.enter_context(tc.tile_pool(name="psS", bufs=1, space="PSUM"))
        psC = ctx.enter_context(tc.tile_pool(name="psC", bufs=1, space="PSUM"))
        psZ = ctx.enter_context(tc.tile_pool(name="psZ", bufs=1, space="PSUM"))

        xt_sb = const.tile([128, NT * 257], f16)
        xc_sb = const.tile([128, 6144], f16)
        wpk_sb = const.tile([128, 578], f32)
        aux_sb = const.tile([1, 514], f32)
        junk_sb = const.tile([128, 256], f16)
        sc0_sb = const.tile([128, 257], f32)
        sc1_sb = const.tile([128, 257], f32)
        S0_sb = const.tile([128, 257], f16)
        S1_sb = const.tile([128, 257], f16)
        SQ0_sb = const.tile([128, 256], f16)
        SQ1_sb = const.tile([128, 256], f16)
        T20_sb = const.tile([128, 256], f16)
        T21_sb = const.tile([128, 256], f16)
        qsxp_sb = const.tile([1, 256], f16)
        b2_sb = const.tile([128, 2], f32)

        wpk16 = wpk_sb.bitcast(f16)   # (128, 1156)
        aux16 = aux_sb.bitcast(f16)   # (1, 1028)
        wQ0 = wpk16[:, 0:256]
        wQ1 = wpk16[:, 256:512]
        wI = wpk16[:, 1024:1152]

        # ---- input DMAs ----
        # xt pieces interleave across both rings with xc halves queued
        # behind them; weights ride the gpsimd SWDGE queue off-ring.
        for c0, pc, ring in XT_PIECES:
            psl = slice(c0 * 257, (c0 + pc) * 257)
            eng = nc.sync if ring == 0 else nc.scalar
            eng.dma_start(out=xt_sb[:, psl], in_=xt[:, psl])
        nc.scalar.dma_start(out=xc_sb[:, 0:3072], in_=xc[:, 0:3072])
        nc.sync.dma_start(out=xc_sb[:, 3072:6144], in_=xc[:, 3072:6144])
        nc.gpsimd.memset(junk_sb, 1.0)
        nc.gpsimd.dma_start(out=wpk_sb, in_=wpk)
        nc.gpsimd.dma_start(out=aux_sb, in_=aux)

        # ---- PE warmup: ramp the clock while the first xt piece lands ----
        wu = psZ.tile([128, 512], f32, tag="z0", name="wu")
        for k in range(N_WARMUP):
            nc.tensor.matmul(
                wu[:, 0:256], lhsT=junk_sb[:, 0:128], rhs=junk_sb,
                start=(k == 0), stop=(k == N_WARMUP - 1),
            )

        # ---- S = X X^T over 24 pixel chunks; col 256 = sx (ones col) ----
        # S0 = S[0:128, 0:257]; S1 only computes S[128:256, 128:257] (129
        # cols) — the S10 block comes later from a PE transpose of S01.
        St0 = psS.tile([128, 512], f32, tag="s0", name="St0")
        St1 = psS.tile([128, 512], f32, tag="s1", name="St1")
        S0 = St0[:, 0:257]
        S1 = St1[:, 0:129]
        for i in range(NT):
            base = i * 257
            nc.tensor.matmul(
                S0, lhsT=xt_sb[:, base : base + 128],
                rhs=xt_sb[:, base : base + 257],
                start=(i == 0), stop=(i == NT - 1),
            )
            nc.tensor.matmul(
                S1, lhsT=xt_sb[:, base + 128 : base + 256],
                rhs=xt_sb[:, base + 128 : base + 257],
                start=(i == 0), stop=(i == NT - 1),
            )
        # S copies. ACT: S0 halves ([128:257] first — it gates more);
        # DVE: the S1 block into S1_sb cols [128:257].
        nc.scalar.copy(out=S0_sb[:, 128:257], in_=S0[:, 128:257])
        nc.vector.tensor_copy(S1_sb[:, 128:257], S1)
        nc.scalar.copy(out=S0_sb[:, 0:128], in_=S0[:, 0:128])

        # ---- chain PSUM tiles (S banks retired by the copies above) ----
        T2t0 = psS.tile([128, 512], f32, tag="s0", name="T2t0")
        T2t1 = psS.tile([128, 512], f32, tag="s1", name="T2t1")
        SQt0 = psZ.tile([128, 512], f32, tag="z0", name="SQt0")
        SQt1 = psZ.tile([128, 512], f32, tag="z1", name="SQt1")

        # S10 = S01^T via PE transpose (PSUM scratch in T2t0 spare cols;
        # transpose output dtype must match its bf16 input)
        s10_p = T2t0[:, 272:336].bitcast(f16)
        nc.tensor.transpose(s10_p, S0_sb[:, 128:256], wI)
        nc.vector.tensor_copy(S1_sb[:, 0:128], s10_p)

        # ---- SQ[c'', c] = sum_c2 S[c2, c''] Q[c2, c] (S symmetric) ----
        # h=1 runs first (its inputs land first); h=0 needs the transpose.
        for h in (1, 0):
            SQt = (SQt0, SQt1)[h]
            hsl = slice(h * 128, (h + 1) * 128)
            nc.tensor.matmul(
                SQt[:, 0:256], lhsT=S0_sb[:, hsl], rhs=wQ0,
                start=True, stop=False,
            )
            nc.tensor.matmul(
                SQt[:, 0:256], lhsT=S1_sb[:, hsl], rhs=wQ1,
                start=False, stop=True,
            )
        # qsx = sx^T Q, psx = sx^T P2  (1, 256) rows
        qsx_p = SQt0[0:1, 256:512]
        nc.tensor.matmul(qsx_p, lhsT=S0_sb[:, 256:257], rhs=wQ0,
                         start=True, stop=False)
        nc.tensor.matmul(qsx_p, lhsT=S1_sb[:, 256:257], rhs=wQ1,
                         start=False, stop=True)

        # copies for the chain (ACT + DVE in parallel, ordered so T2w's
        # inputs land first)
        nc.scalar.copy(out=SQ1_sb, in_=SQt1[:, 0:256])
        nc.vector.scalar_tensor_tensor(   # qsx' = qsx + N hb
            out=qsxp_sb, in0=qsx_p, scalar=1.0, in1=aux16[0:1, 512:768],
            op0=ALU.mult, op1=ALU.add,
        )
        nc.vector.tensor_copy(SQ0_sb, SQt0[:, 0:256])

        # ---- T2w = I + P2^T SQ + tb2 qsx\'^T + psx hb^T  (2 half tiles) ----
        T2ws = (T2t0[:, 0:256], T2t1[:, 0:256])
        for h, T2w in enumerate(T2ws):   # start the group: SQ1 part first
            nc.tensor.matmul(
                T2w, lhsT=wpk16[:, 768 + 128 * h : 896 + 128 * h],
                rhs=SQ1_sb, start=True, stop=False,
            )
        for h, T2w in enumerate(T2ws):   # identity (only needs wpk)
            nc.tensor.matmul(
                T2w[:, 128 * h : 128 * h + 128], lhsT=wI, rhs=wI,
                start=False, stop=False,
            )
        for h, T2w in enumerate(T2ws):   # rank-1: tb2 qsx\'^T
            nc.tensor.matmul(
                T2w, lhsT=aux16[0:1, 128 * h : 128 * h + 128], rhs=qsxp_sb,
                start=False, stop=False,
            )
        for h, T2w in enumerate(T2ws):   # SQ0 part (closes the group)
            nc.tensor.matmul(
                T2w, lhsT=wpk16[:, 512 + 128 * h : 640 + 128 * h],
                rhs=SQ0_sb, start=False, stop=True,
            )
        # b2 = SQ^T t1c + (sx.t1) hb + bc  (two column quarters; the
        # Q^T S t1c term contracts the SQ tiles directly over c2)
        b2ps = (T2t0[:, 264:265], T2t1[:, 264:265])
        for m in range(3):
            for h, b2p in enumerate(b2ps):
                hsl = slice(h * 128, (h + 1) * 128)
                if m == 0:
                    nc.tensor.matmul(b2p, lhsT=SQ0_sb[:, hsl],
                                     rhs=wpk16[:, 1152:1153],
                                     start=True, stop=False)
                elif m == 1:
                    nc.tensor.matmul(b2p, lhsT=SQ1_sb[:, hsl],
                                     rhs=wpk16[:, 1153:1154],
                                     start=False, stop=False)
                else:
                    nc.tensor.matmul(
                        b2p, lhsT=aux16[0:1, 768 + 128 * h : 896 + 128 * h],
                        rhs=aux16[0:1, 1024:1025], start=False, stop=True)

        # T2w copies in quarters: the first output matmul pair needs only
        # cols [0:128] of both halves.
        nc.scalar.copy(out=T20_sb[:, 0:128], in_=T2t0[:, 0:128])
        nc.vector.tensor_copy(T21_sb[:, 0:128], T2t1[:, 0:128])
        nc.scalar.copy(out=T20_sb[:, 128:256], in_=T2t0[:, 128:256])
        nc.vector.tensor_copy(T21_sb[:, 128:256], T2t1[:, 128:256])
        nc.scalar.copy(out=b2_sb[:, 0:1], in_=b2ps[0])
        nc.vector.tensor_copy(b2_sb[:, 1:2], b2ps[1])

        # ---- z[c, n] = sum_c' T2w[c', c] X[c', n] + b2[c]; fp16 out ----
        # Chunk order matches xc arrival (scalar half: 0-2, sync half:
        # 3-5). Per chunk: two PSUM tiles (one per c-half) with matmuls
        # interleaved across banks; ACT assembles half 0 (bias via
        # activation), DVE half 1 (tensor_scalar); one strided DMA per
        # chunk, rings alternating; the last chunk splits across rings.
        zbanks = [("z2", psZ), ("z3", psZ), ("c0", psC), ("c1", psC),
                  ("z0", psZ), ("z1", psZ)]
        for k, j in enumerate(Z_ORDER):
            z_sb = zpool.tile([128, 1024], f16, tag="zs", name=f"z{j}")
            pzs = []
            for hc in range(2):
                tag, pool = zbanks[(2 * k + hc) % 6]
                pz = pool.tile([128, 512], f32, tag=tag, name=f"pz{j}_{hc}")
                pzs.append(pz)
            for m in range(2):
                for hc in range(2):
                    nc.tensor.matmul(
                        pzs[hc],
                        lhsT=(T20_sb if m == 0 else T21_sb)[:, 128 * hc : 128 * hc + 128],
                        rhs=xc_sb[:, j * 1024 + 512 * m : j * 1024 + 512 * m + 512],
                        start=(m == 0), stop=(m == 1),
                    )
            nc.scalar.activation(
                out=z_sb[:, 0:512], in_=pzs[0], func=AF.Identity,
                bias=b2_sb[:, 0:1], scale=1.0,
            )
            nc.vector.tensor_scalar_add(z_sb[:, 512:1024], pzs[1], b2_sb[:, 1:2])
            if k < 5:
                out_ap = bass.AP(
                    tensor=out.tensor, offset=j * 512,
                    ap=[[6144, 128], [3072, 2], [1, 512]],
                )
                eng = nc.sync if k % 2 == 0 else nc.scalar
                eng.dma_start(out=out_ap, in_=z_sb)
            else:
                # last chunk: split across both rings in parallel to
                # shorten the tail that gates the epilogue
                nc.sync.dma_start(
                    out=out[:, j * 512 : (j + 1) * 512], in_=z_sb[:, 0:512]
                )
                nc.scalar.dma_start(
                    out=out[:, 3072 + j * 512 : 3072 + (j + 1) * 512],
                    in_=z_sb[:, 512:1024],
                )

    nc.compile()
    return nc


def _get_nc():
    global _NC
    if _NC is None:
        _NC = _build_nc()
    return _NC


# test.py reads this after a traced run to get exec_time_ns
last_results = None


def _prep_inputs(inputs):
    import ml_dtypes

    f16 = ml_dtypes.bfloat16

    x = np.asarray(inputs["x"], dtype=np.float32)
    theta_w = np.asarray(inputs["theta_w"], np.float64)
    theta_b = np.asarray(inputs["theta_b"], np.float64)
    phi_w = np.asarray(inputs["phi_w"], np.float64)
    phi_b = np.asarray(inputs["phi_b"], np.float64)
    g_w = np.asarray(inputs["g_w"], np.float64)
    g_b = np.asarray(inputs["g_b"], np.float64)
    w_w = np.asarray(inputs["w_w"], np.float64)
    w_b = np.asarray(inputs["w_b"], np.float64)
    bn_gamma = np.asarray(inputs["bn_gamma"], np.float64)
    bn_beta = np.asarray(inputs["bn_beta"], np.float64)
    bn_mean = np.asarray(inputs["bn_mean"], np.float64)
    bn_var = np.asarray(inputs["bn_var"], np.float64)

    inv = bn_gamma / np.sqrt(bn_var + BN_EPS)
    bprime = inv * (w_b - bn_mean) + bn_beta
    wpp = w_w * inv[:, None] / N                              # w'' (C, D)
    Q = g_w.T @ wpp.T                                         # (C, C)
    P2 = phi_w.T @ theta_w                                    # (C, C)
    t1 = phi_w.T @ theta_b
    sbb = float(phi_b @ theta_b)
    t1c = t1 + sbb
    tb2 = theta_w.T @ phi_b
    hb = wpp @ g_b
    bc = N * sbb * hb + bprime

    I128 = np.eye(128, dtype=np.float64)
    tcols = np.zeros((128, 4), np.float64)
    tcols[:, 0] = t1c[0:128]
    tcols[:, 1] = t1c[128:256]
    tcols[:, 2] = t1[0:128]
    tcols[:, 3] = t1[128:256]
    P2q = np.concatenate(
        [P2[0:128, 0:128], P2[0:128, 128:256],
         P2[128:256, 0:128], P2[128:256, 128:256]], axis=1,
    )                                                         # (128, 512)
    wpk_f16 = np.concatenate(
        [Q[0:128], Q[128:256], P2q, I128, tcols], axis=1
    ).astype(f16)                                             # (128, 1156)
    assert wpk_f16.shape == (128, 1156), wpk_f16.shape
    wpk = np.ascontiguousarray(wpk_f16).view(np.uint8).view(np.float32)

    aux_f16 = np.concatenate(
        [tb2, hb, N * hb, bc, [1.0], np.zeros(3)]
    ).astype(f16)                                             # (1028,)
    aux = aux_f16.view(np.uint8).view(np.float32)[None, :]    # (1, 514)

    x16 = x.reshape(B, C, N).astype(f16)
    xt = np.ones((B, NT, 128, 257), f16)
    xt[:, :, :, 0:256] = x16.transpose(0, 2, 1).reshape(B, NT, 128, C)
    xt = np.ascontiguousarray(
        xt.transpose(0, 2, 1, 3).reshape(B, 128, NT * 257)
    )
    xc = np.ascontiguousarray(
        x16.reshape(B, 2, 128, 6, 512).transpose(0, 2, 3, 1, 4).reshape(B, 128, 6144)
    )
    return xt, xc, {"wpk": wpk, "aux": aux}


def kernel(**inputs):
    from concourse.bass_utils import run_bass_kernel_spmd

    global last_results

    xt, xc, shared = _prep_inputs(inputs)
    in_maps = [
        dict(shared, xt=np.ascontiguousarray(xt[b]), xc=np.ascontiguousarray(xc[b]))
        for b in range(B)
    ]

    nc = _get_nc()
    res = run_bass_kernel_spmd(nc, in_maps, list(range(N_CORES)))
    last_results = res

    outs = np.stack([res.results[b]["out"] for b in range(B)])  # (B, 128, 6144)
    z = outs.reshape(B, 128, 2, 3072).transpose(0, 2, 1, 3).reshape(B, C, N)
    return z.reshape(B, C, HH, WW).astype(np.float32)


# revision 16
# speedup vs baseline: 1.0258x; 1.0258x over previous
"""Non-local (dot-product attention) block kernel for Trainium2, 8 cores.

Reference math (per sample):
    t = theta_w @ X + bt 1^T            (D, N)
    p = phi_w @ X + bp 1^T              (D, N)
    g = g_w @ X + bg 1^T                (D, N)
    f = t^T p / N;  y^T = g f^T;  z = BN(w_w y^T) + x

Gram-form collapse (no softmax => pure matmul associativity), with the
D-sized weight chain contracted ON THE HOST into two C x C matrices:
    S   = X X^T, sx = X 1 (ones column)                    (C, C), (C,)
    Q   = g_w^T w''^T,  P2 = phi_w^T theta_w               (host, C x C)
    T2w = I + P2^T (S Q) + tb2 (Q^T sx + N hb)^T + (P2^T sx) hb^T
    b2  = Q^T (S (t1 + sbb)) + (sx . t1) hb + bc           (host vectors)
    z   = T2w^T X + b2 1^T
where w'' = diag(inv) w_w / N, tb2 = theta_w^T bp, hb = w'' bg,
t1 = phi_w^T bt, sbb = bp.bt, bc = N sbb hb + b'.

So the device work is: one C x C x N Gram, a 2-hop C x C chain
(S -> SQ -> T2w, plus tiny row/column bias matmuls), and one C x C x N
output matmul. Everything else is folded on the host.

Device plan per core (data-parallel, one sample per core, no collectives):
  - The PE is power-throttled to ~50% duty under sustained load, so total
    matmul columns are minimized: the Gram exploits symmetry (S1 block
    only computes cols [128:257]; the missing S10 block is one PE
    transpose of S01), and the weight chain is tiny.
  - Per-queue DMA tops out well under the ~360 GB/s core aggregate, so xt
    is interleaved across BOTH HWDGE rings (small first pieces); the xc
    halves queue behind xt, and the packed weights ride the gpsimd SWDGE
    queue so they never block the rings.
  - The PE clock also ramps with activity (~2x slower cold), so a few
    warmup matmuls on junk data run while the first xt piece lands.
  - Output matmuls interleave the two c-half PSUM banks (same-bank
    back-to-back matmuls stall ~2x).
  - PSUM bank budget (8): S banks are reused for T2w + the bias chain
    (disjoint columns), SQ banks carry the qsx/psx rows, and the z phase
    cycles 6 banks (4 dedicated + the retired SQ banks).
"""

import numpy as np

B, C, HH, WW = 8, 256, 96, 32
N = HH * WW          # 3072
D = 128              # inter_channels
BN_EPS = 1e-5
NT = N // 128        # 24 pixel chunks
N_CORES = 8

_NC = None

# xt pieces (chunk_start, n_chunks, ring): ring 0 = sync, ring 1 = scalar
XT_PIECES = [(0, 1, 0), (1, 1, 1), (2, 3, 0), (5, 3, 1),
             (8, 4, 0), (12, 4, 1), (16, 4, 0), (20, 4, 1)]
Z_ORDER = [0, 1, 2, 3, 4, 5]
N_WARMUP = 4


def _build_nc():
    from contextlib import ExitStack

    import concourse.bass as bass
    import concourse.bacc as bacc
    import concourse.tile as tile
    from concourse import mybir

    f32 = mybir.dt.float32
    f16 = mybir.dt.bfloat16
    AF = mybir.ActivationFunctionType
    ALU = mybir.AluOpType

    nc = bacc.Bacc(
        "TRN2",
        target_bir_lowering=False,
        debug=False,
        num_devices=N_CORES,
    )

    # xt: 24 blocks of 257 cols: block i = x^T[128i:128(i+1), :] | ones
    xt = nc.dram_tensor("xt", [128, NT * 257], f16, kind="ExternalInput").ap()
    # xc: 6 blocks of 1024: block j = [x[0:128, 512j:512j+512] | x[128:256, ...]]
    xc = nc.dram_tensor("xc", [128, 6144], f16, kind="ExternalInput").ap()
    # wpk f16 cols: Q0 256 | Q1 256 | P2q 512 | I128 128 | t1c 2 | t1p 2
    wpk = nc.dram_tensor("wpk", [128, 578], f32, kind="ExternalInput").ap()
    # aux f16 cols: tb2 256 | hb 256 | N*hb 256 | bc 256 | one 1 | pad
    aux = nc.dram_tensor("aux", [1, 514], f32, kind="ExternalInput").ap()
    # out: [z[0:128, :] | z[128:256, :]] fp16
    out = nc.dram_tensor("out", [128, 6144], f16, kind="ExternalOutput").ap()

    with tile.TileContext(nc) as tc, ExitStack() as ctx:
        const = ctx.enter_context(tc.tile_pool(name="const", bufs=1))
        zpool = ctx.enter_context(tc.tile_pool(name="zpool", bufs=4))
        psS = ctx.enter_context(tc.tile_pool(name="psS", bufs=1, space="PSUM"))
        psC = ctx.enter_context(tc.tile_pool(name="psC", bufs=1, space="PSUM"))
        psZ = ctx.enter_context(tc.tile_pool(name="psZ", bufs=1, space="PSUM"))

        xt_sb = const.tile([128, NT * 257], f16)
        xc_sb = const.tile([128, 6144], f16)
        wpk_sb = const.tile([128, 578], f32)
        aux_sb = const.tile([1, 514], f32)
        junk_sb = const.tile([128, 256], f16)
        sc0_sb = const.tile([128, 257], f32)
        sc1_sb = const.tile([128, 257], f32)
        S0_sb = const.tile([128, 257], f16)
        S1_sb = const.tile([128, 257], f16)
        SQ0_sb = const.tile([128, 256], f16)
        SQ1_sb = const.tile([128, 256], f16)
        T20_sb = const.tile([128, 256], f16)
        T21_sb = const.tile([128, 256], f16)
        qsxp_sb = const.tile([1, 256], f16)
        b2_sb = const.tile([128, 2], f32)

        wpk16 = wpk_sb.bitcast(f16)   # (128, 1156)
        aux16 = aux_sb.bitcast(f16)   # (1, 1028)
        wQ0 = wpk16[:, 0:256]
        wQ1 = wpk16[:, 256:512]
        wI = wpk16[:, 1024:1152]

        # ---- input DMAs ----
        # xt pieces interleave across both rings with xc halves queued
        # behind them; weights ride the gpsimd SWDGE queue off-ring.
        for c0, pc, ring in XT_PIECES:
            psl = slice(c0 * 257, (c0 + pc) * 257)
            eng = nc.sync if ring == 0 else nc.scalar
            eng.dma_start(out=xt_sb[:, psl], in_=xt[:, psl])
        nc.sync.dma_start(out=wpk_sb, in_=wpk)
        nc.sync.dma_start(out=aux_sb, in_=aux)
        nc.scalar.dma_start(out=xc_sb[:, 0:3072], in_=xc[:, 0:3072])
        nc.sync.dma_start(out=xc_sb[:, 3072:6144], in_=xc[:, 3072:6144])
        nc.gpsimd.memset(junk_sb, 1.0)

        # ---- PE warmup: ramp the clock while the first xt piece lands ----
        wu = psZ.tile([128, 512], f32, tag="z0", name="wu")
        for k in range(N_WARMUP):
            nc.tensor.matmul(
                wu[:, 0:256], lhsT=junk_sb[:, 0:128], rhs=junk_sb,
                start=(k == 0), stop=(k == N_WARMUP - 1),
            )

        # ---- S = X X^T over 24 pixel chunks; col 256 = sx (ones col) ----
        # S0 = S[0:128, 0:257]; S1 only computes S[128:256, 128:257] (129
        # cols) — the S10 block comes later from a PE transpose of S01.
        St0 = psS.tile([128, 512], f32, tag="s0", name="St0")
        St1 = psS.tile([128, 512], f32, tag="s1", name="St1")
        S0 = St0[:, 0:257]
        S1 = St1[:, 0:129]
        for i in range(NT):
            base = i * 257
            nc.tensor.matmul(
                S0, lhsT=xt_sb[:, base : base + 128],
                rhs=xt_sb[:, base : base + 257],
                start=(i == 0), stop=(i == NT - 1),
            )
            nc.tensor.matmul(
                S1, lhsT=xt_sb[:, base + 128 : base + 256],
                rhs=xt_sb[:, base + 128 : base + 257],
                start=(i == 0), stop=(i == NT - 1),
            )
        # S copies. ACT: S0 halves ([128:257] first — it gates more);
        # DVE: the S1 block into S1_sb cols [128:257].
        nc.scalar.copy(out=S0_sb[:, 128:257], in_=S0[:, 128:257])
        nc.vector.tensor_copy(S1_sb[:, 128:257], S1)
        nc.scalar.copy(out=S0_sb[:, 0:128], in_=S0[:, 0:128])

        # ---- chain PSUM tiles (S banks retired by the copies above) ----
        T2t0 = psS.tile([128, 512], f32, tag="s0", name="T2t0")
        T2t1 = psS.tile([128, 512], f32, tag="s1", name="T2t1")
        SQt0 = psZ.tile([128, 512], f32, tag="z0", name="SQt0")
        SQt1 = psZ.tile([128, 512], f32, tag="z1", name="SQt1")

        # S10 = S01^T via PE transpose (PSUM scratch in T2t0 spare cols;
        # transpose output dtype must match its bf16 input)
        s10_p = T2t0[:, 272:336].bitcast(f16)
        nc.tensor.transpose(s10_p, S0_sb[:, 128:256], wI)
        nc.vector.tensor_copy(S1_sb[:, 0:128], s10_p)

        # ---- SQ[c'', c] = sum_c2 S[c2, c''] Q[c2, c] (S symmetric) ----
        # h=1 runs first (its inputs land first); h=0 needs the transpose.
        for h in (1, 0):
            SQt = (SQt0, SQt1)[h]
            hsl = slice(h * 128, (h + 1) * 128)
            nc.tensor.matmul(
                SQt[:, 0:256], lhsT=S0_sb[:, hsl], rhs=wQ0,
                start=True, stop=False,
            )
            nc.tensor.matmul(
                SQt[:, 0:256], lhsT=S1_sb[:, hsl], rhs=wQ1,
                start=False, stop=True,
            )
        # qsx = sx^T Q, psx = sx^T P2  (1, 256) rows
        qsx_p = SQt0[0:1, 256:512]
        nc.tensor.matmul(qsx_p, lhsT=S0_sb[:, 256:257], rhs=wQ0,
                         start=True, stop=False)
        nc.tensor.matmul(qsx_p, lhsT=S1_sb[:, 256:257], rhs=wQ1,
                         start=False, stop=True)

        # copies for the chain (ACT + DVE in parallel, ordered so T2w's
        # inputs land first)
        nc.scalar.copy(out=SQ1_sb, in_=SQt1[:, 0:256])
        nc.vector.scalar_tensor_tensor(   # qsx' = qsx + N hb
            out=qsxp_sb, in0=qsx_p, scalar=1.0, in1=aux16[0:1, 512:768],
            op0=ALU.mult, op1=ALU.add,
        )
        nc.vector.tensor_copy(SQ0_sb, SQt0[:, 0:256])

        # ---- T2w = I + P2^T SQ + tb2 qsx\'^T + psx hb^T  (2 half tiles) ----
        T2ws = (T2t0[:, 0:256], T2t1[:, 0:256])
        for h, T2w in enumerate(T2ws):   # start the group: SQ1 part first
            nc.tensor.matmul(
                T2w, lhsT=wpk16[:, 768 + 128 * h : 896 + 128 * h],
                rhs=SQ1_sb, start=True, stop=False,
            )
        for h, T2w in enumerate(T2ws):   # identity (only needs wpk)
            nc.tensor.matmul(
                T2w[:, 128 * h : 128 * h + 128], lhsT=wI, rhs=wI,
                start=False, stop=False,
            )
        for h, T2w in enumerate(T2ws):   # rank-1: tb2 qsx\'^T
            nc.tensor.matmul(
                T2w, lhsT=aux16[0:1, 128 * h : 128 * h + 128], rhs=qsxp_sb,
                start=False, stop=False,
            )
        for h, T2w in enumerate(T2ws):   # SQ0 part (closes the group)
            nc.tensor.matmul(
                T2w, lhsT=wpk16[:, 512 + 128 * h : 640 + 128 * h],
                rhs=SQ0_sb, start=False, stop=True,
            )
        # b2 = SQ^T t1c + (sx.t1) hb + bc  (two column quarters; the
        # Q^T S t1c term contracts the SQ tiles directly over c2)
        b2ps = (T2t0[:, 264:265], T2t1[:, 264:265])
        for m in range(3):
            for h, b2p in enumerate(b2ps):
                hsl = slice(h * 128, (h + 1) * 128)
                if m == 0:
                    nc.tensor.matmul(b2p, lhsT=SQ0_sb[:, hsl],
                                     rhs=wpk16[:, 1152:1153],
                                     start=True, stop=False)
                elif m == 1:
                    nc.tensor.matmul(b2p, lhsT=SQ1_sb[:, hsl],
                                     rhs=wpk16[:, 1153:1154],
                                     start=False, stop=False)
                else:
                    nc.tensor.matmul(
                        b2p, lhsT=aux16[0:1, 768 + 128 * h : 896 + 128 * h],
                        rhs=aux16[0:1, 1024:1025], start=False, stop=True)

        # T2w copies in quarters: the first output matmul pair needs only
        # cols [0:128] of both halves.
        nc.scalar.copy(out=T20_sb[:, 0:128], in_=T2t0[:, 0:128])
        nc.vector.tensor_copy(T21_sb[:, 0:128], T2t1[:, 0:128])
        nc.scalar.copy(out=T20_sb[:, 128:256], in_=T2t0[:, 128:256])
        nc.vector.tensor_copy(T21_sb[:, 128:256], T2t1[:, 128:256])
        nc.scalar.copy(out=b2_sb[:, 0:1], in_=b2ps[0])
        nc.vector.tensor_copy(b2_sb[:, 1:2], b2ps[1])

        # ---- z[c, n] = sum_c' T2w[c', c] X[c', n] + b2[c]; fp16 out ----
        # Chunk order matches xc arrival (scalar half: 0-2, sync half:
        # 3-5). Per chunk: two PSUM tiles (one per c-half) with matmuls
        # interleaved across banks; ACT assembles half 0 (bias via
        # activation), DVE half 1 (tensor_scalar); one strided DMA per
        # chunk, rings alternating; the last chunk splits across rings.
        zbanks = [("z2", psZ), ("z3", psZ), ("c0", psC), ("c1", psC),
                  ("z0", psZ), ("z1", psZ)]
        for k, j in enumerate(Z_ORDER):
            z_sb = zpool.tile([128, 1024], f16, tag="zs", name=f"z{j}")
            pzs = []
            for hc in range(2):
                tag, pool = zbanks[(2 * k + hc) % 6]
                pz = pool.tile([128, 512], f32, tag=tag, name=f"pz{j}_{hc}")
                pzs.append(pz)
            for m in range(2):
                for hc in range(2):
                    nc.tensor.matmul(
                        pzs[hc],
                        lhsT=(T20_sb if m == 0 else T21_sb)[:, 128 * hc : 128 * hc + 128],
                        rhs=xc_sb[:, j * 1024 + 512 * m : j * 1024 + 512 * m + 512],
                        start=(m == 0), stop=(m == 1),
                    )
            nc.scalar.activation(
                out=z_sb[:, 0:512], in_=pzs[0], func=AF.Identity,
                bias=b2_sb[:, 0:1], scale=1.0,
            )
            nc.vector.tensor_scalar_add(z_sb[:, 512:1024], pzs[1], b2_sb[:, 1:2])
            if k < 5:
                out_ap = bass.AP(
                    tensor=out.tensor, offset=j * 512,
                    ap=[[6144, 128], [3072, 2], [1, 512]],
                )
                eng = nc.sync if k % 2 == 0 else nc.scalar
                eng.dma_start(out=out_ap, in_=z_sb)
            else:
                # last chunk: split across both rings in parallel to
                # shorten the tail that gates the epilogue
                nc.sync.dma_start(
                    out=out[:, j * 512 : (j + 1) * 512], in_=z_sb[:, 0:512]
                )
                nc.scalar.dma_start(
                    out=out[:, 3072 + j * 512 : 3072 + (j + 1) * 512],
                    in_=z_sb[:, 512:1024],
                )

    nc.compile()
    return nc


def _get_nc():
    global _NC
    if _NC is None:
        _NC = _build_nc()
    return _NC


# test.py reads this after a traced run to get exec_time_ns
last_results = None


def _prep_inputs(inputs):
    import ml_dtypes

    f16 = ml_dtypes.bfloat16

    x = np.asarray(inputs["x"], dtype=np.float32)
    theta_w = np.asarray(inputs["theta_w"], np.float64)
    theta_b = np.asarray(inputs["theta_b"], np.float64)
    phi_w = np.asarray(inputs["phi_w"], np.float64)
    phi_b = np.asarray(inputs["phi_b"], np.float64)
    g_w = np.asarray(inputs["g_w"], np.float64)
    g_b = np.asarray(inputs["g_b"], np.float64)
    w_w = np.asarray(inputs["w_w"], np.float64)
    w_b = np.asarray(inputs["w_b"], np.float64)
    bn_gamma = np.asarray(inputs["bn_gamma"], np.float64)
    bn_beta = np.asarray(inputs["bn_beta"], np.float64)
    bn_mean = np.asarray(inputs["bn_mean"], np.float64)
    bn_var = np.asarray(inputs["bn_var"], np.float64)

    inv = bn_gamma / np.sqrt(bn_var + BN_EPS)
    bprime = inv * (w_b - bn_mean) + bn_beta
    wpp = w_w * inv[:, None] / N                              # w'' (C, D)
    Q = g_w.T @ wpp.T                                         # (C, C)
    P2 = phi_w.T @ theta_w                                    # (C, C)
    t1 = phi_w.T @ theta_b
    sbb = float(phi_b @ theta_b)
    t1c = t1 + sbb
    tb2 = theta_w.T @ phi_b
    hb = wpp @ g_b
    bc = N * sbb * hb + bprime

    I128 = np.eye(128, dtype=np.float64)
    tcols = np.zeros((128, 4), np.float64)
    tcols[:, 0] = t1c[0:128]
    tcols[:, 1] = t1c[128:256]
    tcols[:, 2] = t1[0:128]
    tcols[:, 3] = t1[128:256]
    P2q = np.concatenate(
        [P2[0:128, 0:128], P2[0:128, 128:256],
         P2[128:256, 0:128], P2[128:256, 128:256]], axis=1,
    )                                                         # (128, 512)
    wpk_f16 = np.concatenate(
        [Q[0:128], Q[128:256], P2q, I128, tcols], axis=1
    ).astype(f16)                                             # (128, 1156)
    assert wpk_f16.shape == (128, 1156), wpk_f16.shape
    wpk = np.ascontiguousarray(wpk_f16).view(np.uint8).view(np.float32)

    aux_f16 = np.concatenate(
        [tb2, hb, N * hb, bc, [1.0], np.zeros(3)]
    ).astype(f16)                                             # (1028,)
    aux = aux_f16.view(np.uint8).view(np.float32)[None, :]    # (1, 514)

    x16 = x.reshape(B, C, N).astype(f16)
    xt = np.ones((B, NT, 128, 257), f16)
    xt[:, :, :, 0:256] = x16.transpose(0, 2, 1).reshape(B, NT, 128, C)
    xt = np.ascontiguousarray(
        xt.transpose(0, 2, 1, 3).reshape(B, 128, NT * 257)
    )
    xc = np.ascontiguousarray(
        x16.reshape(B, 2, 128, 6, 512).transpose(0, 2, 3, 1, 4).reshape(B, 128, 6144)
    )
    return xt, xc, {"wpk": wpk, "aux": aux}


def kernel(**inputs):
    from concourse.bass_utils import run_bass_kernel_spmd

    global last_results

    xt, xc, shared = _prep_inputs(inputs)
    in_maps = [
        dict(shared, xt=np.ascontiguousarray(xt[b]), xc=np.ascontiguousarray(xc[b]))
        for b in range(B)
    ]

    nc = _get_nc()
    res = run_bass_kernel_spmd(nc, in_maps, list(range(N_CORES)))
    last_results = res

    outs = np.stack([res.results[b]["out"] for b in range(B)])  # (B, 128, 6144)
    z = outs.reshape(B, 128, 2, 3072).transpose(0, 2, 1, 3).reshape(B, C, N)
    return z.reshape(B, C, HH, WW).astype(np.float32)


# revision 17
# speedup vs baseline: 1.0871x; 1.0598x over previous
"""Non-local (dot-product attention) block kernel for Trainium2, 8 cores.

Reference math (per sample):
    t = theta_w @ X + bt 1^T            (D, N)
    p = phi_w @ X + bp 1^T              (D, N)
    g = g_w @ X + bg 1^T                (D, N)
    f = t^T p / N;  y^T = g f^T;  z = BN(w_w y^T) + x

Gram-form collapse (no softmax => pure matmul associativity), with the
D-sized weight chain contracted ON THE HOST into two C x C matrices:
    S   = X X^T, sx = X 1 (ones column)                    (C, C), (C,)
    Q   = g_w^T w''^T,  P2 = phi_w^T theta_w               (host, C x C)
    T2w = I + P2^T (S Q) + tb2 (Q^T sx + N hb)^T + (P2^T sx) hb^T
    b2  = Q^T (S (t1 + sbb)) + (sx . t1) hb + bc           (host vectors)
    z   = T2w^T X + b2 1^T
where w'' = diag(inv) w_w / N, tb2 = theta_w^T bp, hb = w'' bg,
t1 = phi_w^T bt, sbb = bp.bt, bc = N sbb hb + b'.

So the device work is: one C x C x N Gram, a 2-hop C x C chain
(S -> SQ -> T2w, plus tiny row/column bias matmuls), and one C x C x N
output matmul. Everything else is folded on the host.

Device plan per core (data-parallel, one sample per core, no collectives):
  - The PE is power-throttled to ~50% duty under sustained load, so total
    matmul columns are minimized: the Gram exploits symmetry (S1 block
    only computes cols [128:257]; the missing S10 block is one PE
    transpose of S01), and the weight chain is tiny.
  - Per-queue DMA tops out well under the ~360 GB/s core aggregate, so xt
    is interleaved across BOTH HWDGE rings (small first pieces); the xc
    halves queue behind xt, and the packed weights ride the gpsimd SWDGE
    queue so they never block the rings.
  - The PE clock also ramps with activity (~2x slower cold), so a few
    warmup matmuls on junk data run while the first xt piece lands.
  - Output matmuls interleave the two c-half PSUM banks (same-bank
    back-to-back matmuls stall ~2x).
  - PSUM bank budget (8): S banks are reused for T2w + the bias chain
    (disjoint columns), SQ banks carry the qsx/psx rows, and the z phase
    cycles 6 banks (4 dedicated + the retired SQ banks).
"""

import numpy as np

B, C, HH, WW = 8, 256, 96, 32
N = HH * WW          # 3072
D = 128              # inter_channels
BN_EPS = 1e-5
NT = N // 128        # 24 pixel chunks
N_CORES = 8

_NC = None

# xt pieces (chunk_start, n_chunks, ring): ring 0 = sync, ring 1 = scalar
XT_PIECES = [(0, 1, 0), (1, 1, 1), (2, 3, 0), (5, 3, 1),
             (8, 4, 0), (12, 4, 1), (16, 4, 0), (20, 4, 1)]
Z_ORDER = [0, 1, 2, 3, 4, 5]
N_WARMUP = 0


def _build_nc():
    from contextlib import ExitStack

    import concourse.bass as bass
    import concourse.bacc as bacc
    import concourse.tile as tile
    from concourse import mybir

    f32 = mybir.dt.float32
    f16 = mybir.dt.bfloat16
    AF = mybir.ActivationFunctionType
    ALU = mybir.AluOpType

    nc = bacc.Bacc(
        "TRN2",
        target_bir_lowering=False,
        debug=False,
        num_devices=N_CORES,
    )

    # xt: 24 blocks of 257 cols: block i = x^T[128i:128(i+1), :] | ones
    xt = nc.dram_tensor("xt", [128, NT * 257], f16, kind="ExternalInput").ap()
    # xc: 6 blocks of 1024: block j = [x[0:128, 512j:512j+512] | x[128:256, ...]]
    xc = nc.dram_tensor("xc", [128, 6144], f16, kind="ExternalInput").ap()
    # wpk f16 cols: Q0 256 | Q1 256 | P2q 512 | I128 128 | t1c 2 | t1p 2
    wpk = nc.dram_tensor("wpk", [128, 578], f32, kind="ExternalInput").ap()
    # aux f16 cols: tb2 256 | hb 256 | N*hb 256 | bc 256 | one 1 | pad
    aux = nc.dram_tensor("aux", [1, 514], f32, kind="ExternalInput").ap()
    # out: [z[0:128, :] | z[128:256, :]] fp16
    out = nc.dram_tensor("out", [128, 6144], f16, kind="ExternalOutput").ap()

    with tile.TileContext(nc) as tc, ExitStack() as ctx:
        const = ctx.enter_context(tc.tile_pool(name="const", bufs=1))
        zpool = ctx.enter_context(tc.tile_pool(name="zpool", bufs=4))
        psS = ctx.enter_context(tc.tile_pool(name="psS", bufs=1, space="PSUM"))
        psC = ctx.enter_context(tc.tile_pool(name="psC", bufs=1, space="PSUM"))
        psZ = ctx.enter_context(tc.tile_pool(name="psZ", bufs=1, space="PSUM"))

        xt_sb = const.tile([128, NT * 257], f16)
        xc_sb = const.tile([128, 6144], f16)
        wpk_sb = const.tile([128, 578], f32)
        aux_sb = const.tile([1, 514], f32)
        junk_sb = const.tile([128, 256], f16)
        sc0_sb = const.tile([128, 257], f32)
        sc1_sb = const.tile([128, 257], f32)
        S0_sb = const.tile([128, 257], f16)
        S1_sb = const.tile([128, 257], f16)
        SQ0_sb = const.tile([128, 256], f16)
        SQ1_sb = const.tile([128, 256], f16)
        T20_sb = const.tile([128, 256], f16)
        T21_sb = const.tile([128, 256], f16)
        qsxp_sb = const.tile([1, 256], f16)
        b2_sb = const.tile([128, 2], f32)

        wpk16 = wpk_sb.bitcast(f16)   # (128, 1156)
        aux16 = aux_sb.bitcast(f16)   # (1, 1028)
        wQ0 = wpk16[:, 0:256]
        wQ1 = wpk16[:, 256:512]
        wI = wpk16[:, 1024:1152]

        # ---- input DMAs ----
        # xt pieces interleave across both rings with xc halves queued
        # behind them; weights ride the gpsimd SWDGE queue off-ring.
        for c0, pc, ring in XT_PIECES:
            psl = slice(c0 * 257, (c0 + pc) * 257)
            eng = nc.sync if ring == 0 else nc.scalar
            eng.dma_start(out=xt_sb[:, psl], in_=xt[:, psl])
        nc.sync.dma_start(out=wpk_sb, in_=wpk)
        nc.sync.dma_start(out=aux_sb, in_=aux)
        nc.scalar.dma_start(out=xc_sb[:, 0:3072], in_=xc[:, 0:3072])
        nc.sync.dma_start(out=xc_sb[:, 3072:6144], in_=xc[:, 3072:6144])
        nc.gpsimd.memset(junk_sb, 1.0)

        # ---- PE warmup: ramp the clock while the first xt piece lands ----
        wu = psZ.tile([128, 512], f32, tag="z0", name="wu") if N_WARMUP else None
        for k in range(N_WARMUP):
            nc.tensor.matmul(
                wu[:, 0:256], lhsT=junk_sb[:, 0:128], rhs=junk_sb,
                start=(k == 0), stop=(k == N_WARMUP - 1),
            )

        # ---- S = X X^T over 24 pixel chunks; col 256 = sx (ones col) ----
        # S0 = S[0:128, 0:257]; S1 only computes S[128:256, 128:257] (129
        # cols) — the S10 block comes later from a PE transpose of S01.
        St0 = psS.tile([128, 512], f32, tag="s0", name="St0")
        St1 = psS.tile([128, 512], f32, tag="s1", name="St1")
        S0 = St0[:, 0:257]
        S1 = St1[:, 0:129]
        for i in range(NT):
            base = i * 257
            nc.tensor.matmul(
                S0, lhsT=xt_sb[:, base : base + 128],
                rhs=xt_sb[:, base : base + 257],
                start=(i == 0), stop=(i == NT - 1),
            )
            nc.tensor.matmul(
                S1, lhsT=xt_sb[:, base + 128 : base + 256],
                rhs=xt_sb[:, base + 128 : base + 257],
                start=(i == 0), stop=(i == NT - 1),
            )
        # S copies. ACT: S0 halves ([128:257] first — it gates more);
        # DVE: the S1 block into S1_sb cols [128:257].
        nc.scalar.copy(out=S0_sb[:, 128:257], in_=S0[:, 128:257])
        nc.vector.tensor_copy(S1_sb[:, 128:257], S1)
        nc.scalar.copy(out=S0_sb[:, 0:128], in_=S0[:, 0:128])

        # ---- chain PSUM tiles (S banks retired by the copies above) ----
        T2t0 = psS.tile([128, 512], f32, tag="s0", name="T2t0")
        T2t1 = psS.tile([128, 512], f32, tag="s1", name="T2t1")
        SQt0 = psZ.tile([128, 512], f32, tag="z0", name="SQt0")
        SQt1 = psZ.tile([128, 512], f32, tag="z1", name="SQt1")

        # S10 = S01^T via PE transpose (PSUM scratch in T2t0 spare cols;
        # transpose output dtype must match its bf16 input)
        s10_p = T2t0[:, 272:336].bitcast(f16)
        nc.tensor.transpose(s10_p, S0_sb[:, 128:256], wI)
        nc.vector.tensor_copy(S1_sb[:, 0:128], s10_p)

        # ---- SQ[c'', c] = sum_c2 S[c2, c''] Q[c2, c] (S symmetric) ----
        # h=1 runs first (its inputs land first); h=0 needs the transpose.
        for h in (1, 0):
            SQt = (SQt0, SQt1)[h]
            hsl = slice(h * 128, (h + 1) * 128)
            nc.tensor.matmul(
                SQt[:, 0:256], lhsT=S0_sb[:, hsl], rhs=wQ0,
                start=True, stop=False,
            )
            nc.tensor.matmul(
                SQt[:, 0:256], lhsT=S1_sb[:, hsl], rhs=wQ1,
                start=False, stop=True,
            )
        # qsx = sx^T Q, psx = sx^T P2  (1, 256) rows
        qsx_p = SQt0[0:1, 256:512]
        nc.tensor.matmul(qsx_p, lhsT=S0_sb[:, 256:257], rhs=wQ0,
                         start=True, stop=False)
        nc.tensor.matmul(qsx_p, lhsT=S1_sb[:, 256:257], rhs=wQ1,
                         start=False, stop=True)

        # copies for the chain (ACT + DVE in parallel, ordered so T2w's
        # inputs land first)
        nc.scalar.copy(out=SQ1_sb, in_=SQt1[:, 0:256])
        nc.vector.scalar_tensor_tensor(   # qsx' = qsx + N hb
            out=qsxp_sb, in0=qsx_p, scalar=1.0, in1=aux16[0:1, 512:768],
            op0=ALU.mult, op1=ALU.add,
        )
        nc.vector.tensor_copy(SQ0_sb, SQt0[:, 0:256])

        # ---- T2w = I + P2^T SQ + tb2 qsx\'^T + psx hb^T  (2 half tiles) ----
        T2ws = (T2t0[:, 0:256], T2t1[:, 0:256])
        for h, T2w in enumerate(T2ws):   # start the group: SQ1 part first
            nc.tensor.matmul(
                T2w, lhsT=wpk16[:, 768 + 128 * h : 896 + 128 * h],
                rhs=SQ1_sb, start=True, stop=False,
            )
        for h, T2w in enumerate(T2ws):   # identity (only needs wpk)
            nc.tensor.matmul(
                T2w[:, 128 * h : 128 * h + 128], lhsT=wI, rhs=wI,
                start=False, stop=False,
            )
        for h, T2w in enumerate(T2ws):   # rank-1: tb2 qsx\'^T
            nc.tensor.matmul(
                T2w, lhsT=aux16[0:1, 128 * h : 128 * h + 128], rhs=qsxp_sb,
                start=False, stop=False,
            )
        for h, T2w in enumerate(T2ws):   # SQ0 part (closes the group)
            nc.tensor.matmul(
                T2w, lhsT=wpk16[:, 512 + 128 * h : 640 + 128 * h],
                rhs=SQ0_sb, start=False, stop=True,
            )
        # b2 = SQ^T t1c + (sx.t1) hb + bc  (two column quarters; the
        # Q^T S t1c term contracts the SQ tiles directly over c2)
        b2ps = (T2t0[:, 264:265], T2t1[:, 264:265])
        for m in range(3):
            for h, b2p in enumerate(b2ps):
                hsl = slice(h * 128, (h + 1) * 128)
                if m == 0:
                    nc.tensor.matmul(b2p, lhsT=SQ0_sb[:, hsl],
                                     rhs=wpk16[:, 1152:1153],
                                     start=True, stop=False)
                elif m == 1:
                    nc.tensor.matmul(b2p, lhsT=SQ1_sb[:, hsl],
                                     rhs=wpk16[:, 1153:1154],
                                     start=False, stop=False)
                else:
                    nc.tensor.matmul(
                        b2p, lhsT=aux16[0:1, 768 + 128 * h : 896 + 128 * h],
                        rhs=aux16[0:1, 1024:1025], start=False, stop=True)

        # T2w copies in quarters: the first output matmul pair needs only
        # cols [0:128] of both halves.
        nc.scalar.copy(out=T20_sb[:, 0:128], in_=T2t0[:, 0:128])
        nc.vector.tensor_copy(T21_sb[:, 0:128], T2t1[:, 0:128])
        nc.scalar.copy(out=T20_sb[:, 128:256], in_=T2t0[:, 128:256])
        nc.vector.tensor_copy(T21_sb[:, 128:256], T2t1[:, 128:256])
        nc.scalar.copy(out=b2_sb[:, 0:1], in_=b2ps[0])
        nc.vector.tensor_copy(b2_sb[:, 1:2], b2ps[1])

        # ---- z[c, n] = sum_c' T2w[c', c] X[c', n] + b2[c]; fp16 out ----
        # Chunk order matches xc arrival (scalar half: 0-2, sync half:
        # 3-5). Per chunk: two PSUM tiles (one per c-half) with matmuls
        # interleaved across banks; ACT assembles half 0 (bias via
        # activation), DVE half 1 (tensor_scalar); one strided DMA per
        # chunk, rings alternating; the last chunk splits across rings.
        zbanks = [("z2", psZ), ("z3", psZ), ("c0", psC), ("c1", psC),
                  ("z0", psZ), ("z1", psZ)]
        for k, j in enumerate(Z_ORDER):
            z_sb = zpool.tile([128, 1024], f16, tag="zs", name=f"z{j}")
            pzs = []
            for hc in range(2):
                tag, pool = zbanks[(2 * k + hc) % 6]
                pz = pool.tile([128, 512], f32, tag=tag, name=f"pz{j}_{hc}")
                pzs.append(pz)
            for m in range(2):
                for hc in range(2):
                    nc.tensor.matmul(
                        pzs[hc],
                        lhsT=(T20_sb if m == 0 else T21_sb)[:, 128 * hc : 128 * hc + 128],
                        rhs=xc_sb[:, j * 1024 + 512 * m : j * 1024 + 512 * m + 512],
                        start=(m == 0), stop=(m == 1),
                    )
            nc.scalar.activation(
                out=z_sb[:, 0:512], in_=pzs[0], func=AF.Identity,
                bias=b2_sb[:, 0:1], scale=1.0,
            )
            nc.vector.tensor_scalar_add(z_sb[:, 512:1024], pzs[1], b2_sb[:, 1:2])
            if k < 5:
                out_ap = bass.AP(
                    tensor=out.tensor, offset=j * 512,
                    ap=[[6144, 128], [3072, 2], [1, 512]],
                )
                eng = nc.sync if k % 2 == 0 else nc.scalar
                eng.dma_start(out=out_ap, in_=z_sb)
            else:
                # last chunk: split across both rings in parallel to
                # shorten the tail that gates the epilogue
                nc.sync.dma_start(
                    out=out[:, j * 512 : (j + 1) * 512], in_=z_sb[:, 0:512]
                )
                nc.scalar.dma_start(
                    out=out[:, 3072 + j * 512 : 3072 + (j + 1) * 512],
                    in_=z_sb[:, 512:1024],
                )

    nc.compile()
    return nc


def _get_nc():
    global _NC
    if _NC is None:
        _NC = _build_nc()
    return _NC


# test.py reads this after a traced run to get exec_time_ns
last_results = None


def _prep_inputs(inputs):
    import ml_dtypes

    f16 = ml_dtypes.bfloat16

    x = np.asarray(inputs["x"], dtype=np.float32)
    theta_w = np.asarray(inputs["theta_w"], np.float64)
    theta_b = np.asarray(inputs["theta_b"], np.float64)
    phi_w = np.asarray(inputs["phi_w"], np.float64)
    phi_b = np.asarray(inputs["phi_b"], np.float64)
    g_w = np.asarray(inputs["g_w"], np.float64)
    g_b = np.asarray(inputs["g_b"], np.float64)
    w_w = np.asarray(inputs["w_w"], np.float64)
    w_b = np.asarray(inputs["w_b"], np.float64)
    bn_gamma = np.asarray(inputs["bn_gamma"], np.float64)
    bn_beta = np.asarray(inputs["bn_beta"], np.float64)
    bn_mean = np.asarray(inputs["bn_mean"], np.float64)
    bn_var = np.asarray(inputs["bn_var"], np.float64)

    inv = bn_gamma / np.sqrt(bn_var + BN_EPS)
    bprime = inv * (w_b - bn_mean) + bn_beta
    wpp = w_w * inv[:, None] / N                              # w'' (C, D)
    Q = g_w.T @ wpp.T                                         # (C, C)
    P2 = phi_w.T @ theta_w                                    # (C, C)
    t1 = phi_w.T @ theta_b
    sbb = float(phi_b @ theta_b)
    t1c = t1 + sbb
    tb2 = theta_w.T @ phi_b
    hb = wpp @ g_b
    bc = N * sbb * hb + bprime

    I128 = np.eye(128, dtype=np.float64)
    tcols = np.zeros((128, 4), np.float64)
    tcols[:, 0] = t1c[0:128]
    tcols[:, 1] = t1c[128:256]
    tcols[:, 2] = t1[0:128]
    tcols[:, 3] = t1[128:256]
    P2q = np.concatenate(
        [P2[0:128, 0:128], P2[0:128, 128:256],
         P2[128:256, 0:128], P2[128:256, 128:256]], axis=1,
    )                                                         # (128, 512)
    wpk_f16 = np.concatenate(
        [Q[0:128], Q[128:256], P2q, I128, tcols], axis=1
    ).astype(f16)                                             # (128, 1156)
    assert wpk_f16.shape == (128, 1156), wpk_f16.shape
    wpk = np.ascontiguousarray(wpk_f16).view(np.uint8).view(np.float32)

    aux_f16 = np.concatenate(
        [tb2, hb, N * hb, bc, [1.0], np.zeros(3)]
    ).astype(f16)                                             # (1028,)
    aux = aux_f16.view(np.uint8).view(np.float32)[None, :]    # (1, 514)

    x16 = x.reshape(B, C, N).astype(f16)
    xt = np.ones((B, NT, 128, 257), f16)
    xt[:, :, :, 0:256] = x16.transpose(0, 2, 1).reshape(B, NT, 128, C)
    xt = np.ascontiguousarray(
        xt.transpose(0, 2, 1, 3).reshape(B, 128, NT * 257)
    )
    xc = np.ascontiguousarray(
        x16.reshape(B, 2, 128, 6, 512).transpose(0, 2, 3, 1, 4).reshape(B, 128, 6144)
    )
    return xt, xc, {"wpk": wpk, "aux": aux}


def kernel(**inputs):
    from concourse.bass_utils import run_bass_kernel_spmd

    global last_results

    xt, xc, shared = _prep_inputs(inputs)
    in_maps = [
        dict(shared, xt=np.ascontiguousarray(xt[b]), xc=np.ascontiguousarray(xc[b]))
        for b in range(B)
    ]

    nc = _get_nc()
    res = run_bass_kernel_spmd(nc, in_maps, list(range(N_CORES)))
    last_results = res

    outs = np.stack([res.results[b]["out"] for b in range(B)])  # (B, 128, 6144)
    z = outs.reshape(B, 128, 2, 3072).transpose(0, 2, 1, 3).reshape(B, C, N)
    return z.reshape(B, C, HH, WW).astype(np.float32)
